# revision 25
# baseline (speedup 1.0000x reference)
"""CFConv (gnn message passing) Trainium2 kernel.

Math (per batch b):
    f1 = ssp(r @ W1 + b1)            ssp(x) = softplus(x) - log2
    f2 = ssp(f1 @ W2 + b2)
    out[i, d] = sum_j x[j, d] * f2[i, j, d]

Sharding: data-parallel over batch B=8 across the 8 cores (one batch each).

softplus is evaluated in ONE ACT pass + one fused DVE op per layer via the
variational (Legendre) form
    softplus(z) = z*sigma(z) + S(sigma(z)),
    S(p) = -p ln p - (1-p) ln(1-p)   (binary entropy),
which is first-order INSENSITIVE to errors in p = sigma(z) (d/dp vanishes
at p = sigma(z)), so a bf16 p and an approximate S are both safe.  S is
fitted as
    S(p) ~= d + w*(a + c*z^2),  w = p*(1-p)
(the w*z^2 term captures the -w*ln w tails since z ~= -+ln w there).  The
constant d costs nothing on device: layer-1's d folds into layer-2's bias
row and layer-2's d folds into the output correction vector.

Why this beats the classic Exp->Ln two-ACT-pass softplus here: measured on
hardware, an ACT op costs ~1.15us/1024 cols and the fused DVE op ~1.47us,
so sigma+finish = 1.15 ACT + 1.47 DVE per 1024 cols per layer, splitting
the work across two engines (ACT ~147us, DVE ~188us per core) instead of
piling 4 passes on ACT (~260us).  Mixing the two recipes is impossible:
Sigmoid shares no ACT table set with Exp/Ln, and a table switch costs
~2.7us.

The fused DVE op (8 ALU stages, the hardware max):
    out = z*p + w*(C0 + C2*z^2)      in0=z (f32, PSUM), in1=p (bf16, SBUF)
registered at import into dve_ops.OPS; the per-NEFF DVE table mechanism
ships it to the device (no firmware change).

Bias handling (keeps all three custom-op scalar slots free for fit consts):
 - layer 1: b1 (~0.03) enters sigma via the ACT affine; its silu-term
   effect b1*p is folded host-side as 0.5*b1^T W2 into the layer-2 bias
   (residual ~(p-0.5)*b1 is negligible).  mm1 output stays raw.
 - layer 2: bias reaches PSUM exactly via a K=2 ones matmul whose
   stationary holds the bias split hi/lo across two bf16 rows (~4e-6).
End-to-end sim error of the full bf16 pipeline vs the f64 reference:
~1.2e-2 (gate 2e-2).

Per-core pipeline (features on partitions, (i,j)-pairs on the free dim):
  r pairs DMA-transposed to SBUF [128, pairs] bf16 (even j in partitions
  0:64, odd in 64:128), chunked 4096 pairs.  Both layers work in 1024-col
  PSUM half-groups (2 banks, double-buffered pools: 2+2 tiles = 8 banks),
  so the in-order engine queues pipeline with two steps of slack.  Layer-2
  work of chunk c-1 interleaves with layer-1 of chunk c.

The f2*x products and the whole j-reduction (7-level f32 add tree +
even/odd combine + corr) run on the otherwise-idle GPSIMD (Pool) engine,
keeping the DVE queue free for the softplus-finish ops that gate PSUM
reuse.  Output stays [d, i] on device; the host transposes back.
"""

import numpy as np
import ml_dtypes

import concourse.bass as bass
import concourse.tile as tile
from concourse import bacc, mybir
from concourse.bass_utils import run_bass_kernel_spmd

LOG2 = float(np.log(2.0))

B, N, D, RBF = 8, 256, 128, 64
PAIRS = N * N // 2            # 32768 row-pairs per batch
CHUNK_PAIRS = 4096            # pairs per DMA-transpose chunk (1 MiB)
GROUP_PAIRS = 1024            # pairs per half-group (8 query nodes i)
SUB = 512                     # cols per matmul (one PSUM bank)
HG = 1024                     # cols per PSUM half-group tile
I_PER_GROUP = GROUP_PAIRS // (N // 2)   # 8
H = CHUNK_PAIRS // GROUP_PAIRS          # groups per chunk tile (4)
N_CORES = 8

BF16 = mybir.dt.bfloat16
F32 = mybir.dt.float32

# Entropy-term fit constants: S(p) ~= d + a*w + c*w*z^2, w = p(1-p).
# Minimax-fitted over the empirical preactivation ranges (layer1 z ~ +-7,
# layer2 z ~ +-4.5); the d's are folded host-side (see module docstring).
D1_C, A1_C, C1_C = -0.00505643, 2.80592749, 0.17756259
D2_C, A2_C, C2_C = -0.01382355, 2.83344796, 0.19336128

_SOFTPLUS_OP = None


def _register_softplus_op():
    """Register the fused softplus-finish DVE op (idempotent).

    out = in0*in1 + w*(s0 + imm2*in0^2),  w = in1*(1-in1)
    """
    global _SOFTPLUS_OP
    if _SOFTPLUS_OP is not None:
        return _SOFTPLUS_OP
    import concourse.dve_ops as dve_ops

    name = "SOFTPLUS_VAR_FIN"
    for op in dve_ops.OPS:
        if op.name == name:
            _SOFTPLUS_OP = op
            return op

    from concourse.dve_ops import DveOp
    from concourse.dve_spec import C0, C2, One, Spec, Src0, Src1, lower, sq
    from concourse.dve_uop import DveOpSpec

    w = Src1 * (One - Src1)
    body = w * (sq(Src0) * C2 + C0) + Src0 * Src1

    def ref(in0, in1, s0, s1, imm2):
        in0 = np.asarray(in0, np.float32)
        in1 = np.asarray(in1, np.float32)
        wv = in1 * (1.0 - in1)
        return wv * (in0 * in0 * imm2 + s0) + in0 * in1

    spec = Spec(body=body, reference=ref)
    row = dve_ops._CUSTOM_DVE_ROW_BASE + len(dve_ops.OPS)
    shas = {}
    for ver in ("v3", "v4"):
        uops = lower(spec, ver=ver)
        shas[ver] = DveOpSpec(
            name=name, opcode=row, uops=uops, rd1_en=True
        ).sha(ver)
    op = DveOp(name, spec, subdim=False, uops_sha=shas)
    dve_ops.OPS.append(op)
    dve_ops._SUB_OPCODE_FOR_NAME[name] = row
    dve_ops.CUSTOM_DVE_SPECS[name] = spec
    _SOFTPLUS_OP = op
    return op


def _build_program(reps: int = 1, unroll: int = 1):
    sp_op = _register_softplus_op()

    nc = bacc.Bacc("TRN2", target_bir_lowering=False, debug=False,
                   num_devices=N_CORES)

    rp = nc.dram_tensor("rp", [PAIRS, 2 * RBF], BF16, kind="ExternalInput").ap()
    xte = nc.dram_tensor("xte", [D, N // 2], BF16, kind="ExternalInput").ap()
    xto = nc.dram_tensor("xto", [D, N // 2], BF16, kind="ExternalInput").ap()
    corr = nc.dram_tensor("corr", [D, 1], F32, kind="ExternalInput").ap()
    w1s = nc.dram_tensor("w1s", [2 * RBF, D], BF16, kind="ExternalInput").ap()
    w2 = nc.dram_tensor("w2", [D, D], BF16, kind="ExternalInput").ap()
    b1c = nc.dram_tensor("b1c", [D, 1], F32, kind="ExternalInput").ap()
    bias2 = nc.dram_tensor("bias2", [2, D], BF16, kind="ExternalInput").ap()
    ones = nc.dram_tensor("ones", [2, SUB], BF16, kind="ExternalInput").ap()
    outT = nc.dram_tensor("outT", [D, N], F32, kind="ExternalOutput").ap()

    f_sig = mybir.ActivationFunctionType.Sigmoid
    mult = mybir.AluOpType.mult

    with tile.TileContext(nc) as tc:
        with (
            tc.tile_pool(name="const", bufs=1) as const,
            tc.tile_pool(name="rt", bufs=3) as rt_pool,
            tc.tile_pool(name="p1", bufs=2) as p1_pool,
            tc.tile_pool(name="p2", bufs=2) as p2_pool,
            tc.tile_pool(name="a1", bufs=2) as a1_pool,
            tc.tile_pool(name="f2", bufs=2) as f2_pool,
            tc.tile_pool(name="prod", bufs=2) as prod_pool,
            tc.tile_pool(name="tree", bufs=1) as tree_pool,
            tc.tile_pool(name="acc", bufs=2) as acc_pool,
            tc.tile_pool(name="osb", bufs=1) as out_pool,
            tc.tile_pool(name="z1", bufs=2, space="PSUM") as z1_pool,
            tc.tile_pool(name="z2", bufs=2, space="PSUM") as z2_pool,
        ):
            w1s_t = const.tile([2 * RBF, D], BF16, tag="w1s")
            w2_t = const.tile([D, D], BF16, tag="w2")
            xte_t = const.tile([D, N // 2], BF16, tag="xte")
            xto_t = const.tile([D, N // 2], BF16, tag="xto")
            b1_t = const.tile([D, 1], F32, tag="b1")
            bias2_t = const.tile([2, D], BF16, tag="bias2")
            ones_t = const.tile([2, SUB], BF16, tag="ones")
            corr_t = const.tile([D, 1], F32, tag="corr")
            nc.sync.dma_start(w1s_t[:], w1s[:])
            nc.sync.dma_start(b1_t[:], b1c[:])
            nc.sync.dma_start(w2_t[:], w2[:])
            nc.sync.dma_start(xte_t[:], xte[:])
            nc.sync.dma_start(xto_t[:], xto[:])
            nc.sync.dma_start(bias2_t[:], bias2[:])
            nc.sync.dma_start(ones_t[:], ones[:])
            nc.sync.dma_start(corr_t[:], corr[:])

            out_sb = out_pool.tile([D, N], F32, tag="osb")

            # Tiny warmup activation right after the const loads: hoists the
            # ~2.7us ACT table load to t~0 where it overlaps the first DMA.
            warm = acc_pool.tile([D, 1], F32, tag="warm")
            nc.scalar.activation(warm[:], b1_t[:], f_sig, bias=0.0)

            jw = N // 2
            PW = H * 2 * HG               # z-cols per chunk tile (8192)
            I_PAIR = H * I_PER_GROUP      # 32 query nodes per chunk tile
            M = PW // jw                  # reduce segments per chunk (64)

            def stage1_half(rt, a1w, hh):
                """mm1 -> sigmoid(+b1) -> softplus-finish for one 1024-col
                half-group (one j-parity of one group) of the current chunk.

                Half-groups + double-buffered 2-bank PSUM pools give the
                mm -> sigma -> finish chains two steps of slack, so the
                in-order engine queues pipeline instead of serializing."""
                h, par = hh // 2, hh % 2
                g0 = h * GROUP_PAIRS
                r0, r1 = par * RBF, (par + 1) * RBF
                z1 = z1_pool.tile([D, HG], F32, tag="z1")
                for s in range(HG // SUB):
                    cs = g0 + s * SUB
                    nc.tensor.matmul(
                        z1[:, s * SUB:(s + 1) * SUB],
                        w1s_t[r0:r1, :],
                        rt[r0:r1, cs:cs + SUB],
                    )
                p1 = p1_pool.tile([D, HG], BF16, tag="p1")
                nc.scalar.activation(p1[:], z1[:], f_sig, bias=b1_t[:])
                nc.vector._custom_dve(
                    sp_op, out=a1w[:, hh * HG:(hh + 1) * HG], in0=z1[:],
                    in1=p1[:], s0=A1_C, imm2=C1_C,
                )

            def stage2_half(a1w, f2w, prod, hh):
                """bias+mm2 -> sigmoid -> softplus-finish -> *x for one
                1024-col half-group of the previous chunk.  The f2*x product
                runs on the Pool engine."""
                c0 = hh * HG
                z2 = z2_pool.tile([D, HG], F32, tag="z2")
                for s in range(HG // SUB):
                    nc.tensor.matmul(
                        z2[:, s * SUB:(s + 1) * SUB],
                        bias2_t[:], ones_t[:],
                        start=True, stop=False, skip_group_check=True,
                    )
                for s in range(HG // SUB):
                    nc.tensor.matmul(
                        z2[:, s * SUB:(s + 1) * SUB],
                        w2_t[:],
                        a1w[:, c0 + s * SUB:c0 + (s + 1) * SUB],
                        start=False, stop=True, skip_group_check=True,
                    )
                p2 = p2_pool.tile([D, HG], BF16, tag="p2")
                nc.scalar.activation(p2[:], z2[:], f_sig, bias=0.0)
                nc.vector._custom_dve(
                    sp_op, out=f2w[:, c0:c0 + HG], in0=z2[:],
                    in1=p2[:], s0=A2_C, imm2=C2_C,
                )
                xb = (xte_t if hh % 2 == 0 else xto_t)[:, None, :]
                nc.gpsimd.tensor_tensor(
                    prod[:, c0:c0 + HG].rearrange("p (k j) -> p k j", j=jw),
                    f2w[:, c0:c0 + HG].rearrange("p (k j) -> p k j", j=jw),
                    xb.broadcast_to([D, I_PER_GROUP, jw]),
                    mult,
                )

            def chunk_tail(prod, i0):
                """j-reduction of one chunk's f2*x products: a 7-level f32
                tensor_add tree + even/odd combine + corr add, all on the
                Pool engine."""
                m3 = prod[:].rearrange("p (m j) -> p m j", j=jw)
                t = m3
                for lvl in range(7):
                    half = jw >> (lvl + 1)
                    nxt = tree_pool.tile([D, M, half], F32, tag=f"t{lvl}")
                    nc.gpsimd.tensor_add(
                        nxt[:], t[:, :, 0:half], t[:, :, half:2 * half])
                    t = nxt
                # t is [D, M, 1]; segments m = h*16 + par*8 + k
                s4 = t[:].rearrange(
                    "p (h par k) o -> p h par (k o)", h=H, par=2)
                tmp = acc_pool.tile([D, I_PAIR], F32, tag="tmp")
                nc.gpsimd.tensor_add(
                    tmp[:].rearrange("p (h k) -> p h k", h=H),
                    s4[:, :, 0, :], s4[:, :, 1, :])
                nc.gpsimd.tensor_scalar_add(
                    out_sb[:, i0:i0 + I_PAIR], tmp[:], corr_t[:])

            def group_tail(prod, g, i0):
                """per-group variant of chunk_tail used in the final flush so
                the tree overlaps the remaining layer-2 work."""
                m3 = prod[:].rearrange("p (m j) -> p m j", j=jw)
                t = m3[:, 16 * g:16 * (g + 1), :]
                for lvl in range(7):
                    half = jw >> (lvl + 1)
                    nxt = tree_pool.tile([D, 16, half], F32, tag=f"g{lvl}")
                    nc.gpsimd.tensor_add(
                        nxt[:], t[:, :, 0:half], t[:, :, half:2 * half])
                    t = nxt
                s4 = t[:].rearrange("p (par k) o -> p par (k o)", par=2)
                tmp = acc_pool.tile([D, I_PER_GROUP], F32, tag="tmpg")
                nc.gpsimd.tensor_add(tmp[:], s4[:, 0, :], s4[:, 1, :])
                nc.gpsimd.tensor_scalar_add(
                    out_sb[:, i0 + 8 * g:i0 + 8 * (g + 1)], tmp[:], corr_t[:])

            # Software-pipelined emission interleaving half-groups of chunk
            # c's layer 1 with half-groups of chunk c-1's layer 2.
            def body():
                pending = None  # (a1w, i0) of the previous chunk
                for c in range(PAIRS // CHUNK_PAIRS):
                    rt = rt_pool.tile([2 * RBF, CHUNK_PAIRS], BF16, tag="rt")
                    if c == 0:
                        # Slice the first transpose 8 ways so mm1 of the
                        # first half-group starts as soon as 128 KiB lands.
                        qq = CHUNK_PAIRS // 8
                        for k in range(8):
                            nc.sync.dma_start_transpose(
                                out=rt[:, k * qq:(k + 1) * qq],
                                in_=rp[k * qq:(k + 1) * qq, :],
                            )
                    else:
                        nc.sync.dma_start_transpose(
                            out=rt[:],
                            in_=rp[c * CHUNK_PAIRS:(c + 1) * CHUNK_PAIRS, :],
                        )
                    a1w = a1_pool.tile([D, PW], BF16, tag="a1")
                    if pending is not None:
                        f2w = f2_pool.tile([D, PW], BF16, tag="f2")
                        prod = prod_pool.tile([D, PW], BF16, tag="prod")
                    for h in range(H):
                        stage1_half(rt, a1w, 2 * h)
                        if pending is not None:
                            stage2_half(pending[0], f2w, prod, 2 * h)
                        stage1_half(rt, a1w, 2 * h + 1)
                        if pending is not None:
                            stage2_half(pending[0], f2w, prod, 2 * h + 1)
                    if pending is not None:
                        chunk_tail(prod, pending[1])
                    pending = (a1w, c * I_PAIR)
                # flush the last chunk's layer 2 (z2 pool double-buffers);
                # per-group tails so the Pool tree overlaps the layer-2 work.
                f2w = f2_pool.tile([D, PW], BF16, tag="f2")
                prod = prod_pool.tile([D, PW], BF16, tag="prod")
                for hh in range(2 * H):
                    stage2_half(pending[0], f2w, prod, hh)
                    if hh % 2 == 1:
                        group_tail(prod, hh // 2, pending[1])

            if unroll > 1:
                for _ in range(unroll):
                    body()
            elif reps == 1:
                body()
            else:
                with tc.For_i(0, reps, 1):
                    body()

            nc.sync.dma_start(outT[:], out_sb[:])

    nc.compile()
    return nc


def _prepare_inputs(x, r, W1, b1, W2, b2):
    bf16 = ml_dtypes.bfloat16
    W1 = np.asarray(W1, np.float32)
    W2 = np.asarray(W2, np.float32)
    W2d = W2.astype(np.float64)
    b1d = np.asarray(b1, np.float64)
    w1s = np.concatenate([W1, W1], axis=0).astype(bf16)          # [128, 128]
    w2b = W2.astype(bf16)                                        # [128, 128]
    # layer-2 bias with: the -log2 shift of layer 1's ssp, layer-1's fit
    # constant d1 (both through W2's column sums), and the mean effect of
    # b1's silu term (0.5 * b1^T W2; bias1 has no PSUM matmul of its own).
    b2n = (np.asarray(b2, np.float64)
           + (D1_C - LOG2) * W2d.sum(axis=0)
           + 0.5 * (b1d @ W2d))

    hi = b2n.astype(np.float32).astype(bf16)
    lo = (b2n - hi.astype(np.float64)).astype(np.float32).astype(bf16)
    bias2 = np.stack([hi, lo])                                   # [2, D]
    ones = np.ones((2, SUB), dtype=bf16)
    b1c = b1d.astype(np.float32).reshape(D, 1)

    in_maps = []
    for b in range(B):
        xbT = np.asarray(x[b], np.float32).T                     # [128 d, 256 j]
        in_maps.append({
            "rp": np.ascontiguousarray(
                np.asarray(r[b], np.float32).reshape(PAIRS, 2 * RBF)
            ).astype(bf16),
            "xte": np.ascontiguousarray(xbT[:, 0::2]).astype(bf16),
            "xto": np.ascontiguousarray(xbT[:, 1::2]).astype(bf16),
            # layer-2's -log2 shift and fit constant d2, scaled by sum_j x.
            "corr": ((D2_C - LOG2) * xbT.sum(axis=1, dtype=np.float64)
                     ).astype(np.float32).reshape(D, 1),
            "w1s": w1s,
            "w2": w2b,
            "b1c": b1c,
            "bias2": bias2,
            "ones": ones,
        })
    return in_maps


_NC_CACHE = None


def _get_nc():
    global _NC_CACHE
    if _NC_CACHE is None:
        _NC_CACHE = _build_program()
    return _NC_CACHE


def hw_time_ns(inputs, reps=2049, n_meas=4):
    """Measure on-device per-iteration time by comparing wall time of a
    reps-times device loop against a single-iteration run."""
    import time as _time
    in_maps = _prepare_inputs(**inputs)

    def run_with(nc_prog):
        ts = []
        for _ in range(n_meas):
            t0 = _time.time()
            run_bass_kernel_spmd(nc_prog, in_maps, list(range(N_CORES)))
            ts.append(_time.time() - t0)
        return min(ts)

    nc1 = _build_program(reps=1)
    ncr = _build_program(reps=reps)
    w1 = run_with(nc1)
    wr = run_with(ncr)
    return (wr - w1) / (reps - 1) * 1e9


def kernel(x, r, W1, b1, W2, b2, _trace=False, _trace_kwargs=None):
    nc = _get_nc()
    in_maps = _prepare_inputs(x, r, W1, b1, W2, b2)
    res = run_bass_kernel_spmd(
        nc, in_maps, list(range(N_CORES)),
        trace=_trace, **(_trace_kwargs or {}),
    )
    out = np.stack([
        np.asarray(res.results[b]["outT"], np.float32).T for b in range(B)
    ])
    if _trace:
        return out, res
    return out


# revision 26
# speedup vs baseline: 1.1540x; 1.1540x over previous
"""CFConv (gnn message passing) Trainium2 kernel.

Math (per batch b):
    f1 = ssp(r @ W1 + b1)            ssp(x) = softplus(x) - log2
    f2 = ssp(f1 @ W2 + b2)
    out[i, d] = sum_j x[j, d] * f2[i, j, d]

Sharding: data-parallel over batch B=8 across the 8 cores (one batch each).

softplus is evaluated in ONE ACT pass + one fused DVE op per layer via the
variational (Legendre) form
    softplus(z) = z*sigma(z) + S(sigma(z)),
    S(p) = -p ln p - (1-p) ln(1-p)   (binary entropy),
which is first-order INSENSITIVE to errors in p = sigma(z) (d/dp vanishes
at p = sigma(z)), so a bf16 p and an approximate S are both safe.  S is
fitted as
    S(p) ~= d + w*(a + c*z^2),  w = p*(1-p)
(the w*z^2 term captures the -w*ln w tails since z ~= -+ln w there).  The
constant d costs nothing on device: layer-1's d folds into layer-2's bias
row and layer-2's d folds into the output correction vector.

Why this beats the classic Exp->Ln two-ACT-pass softplus here: measured on
hardware, an ACT op costs ~1.15us/1024 cols and the fused DVE op ~1.47us,
so sigma+finish = 1.15 ACT + 1.47 DVE per 1024 cols per layer, splitting
the work across two engines (ACT ~147us, DVE ~188us per core) instead of
piling 4 passes on ACT (~260us).  Mixing the two recipes is impossible:
Sigmoid shares no ACT table set with Exp/Ln, and a table switch costs
~2.7us.

The fused DVE op (8 ALU stages, the hardware max):
    out = z*p + w*(C0 + C2*z^2)      in0=z (f32, PSUM), in1=p (bf16, SBUF)
registered at import into dve_ops.OPS; the per-NEFF DVE table mechanism
ships it to the device (no firmware change).

Bias handling (keeps all three custom-op scalar slots free for fit consts):
 - layer 1: b1 (~0.03) enters sigma via the ACT affine; its silu-term
   effect b1*p is folded host-side as 0.5*b1^T W2 into the layer-2 bias
   (residual ~(p-0.5)*b1 is negligible).  mm1 output stays raw.
 - layer 2: bias reaches PSUM exactly via a K=2 ones matmul whose
   stationary holds the bias split hi/lo across two bf16 rows (~4e-6).
End-to-end sim error of the full bf16 pipeline vs the f64 reference:
~1.2e-2 (gate 2e-2).

Per-core pipeline (features on partitions, (i,j)-pairs on the free dim):
  r pairs DMA-transposed to SBUF [128, pairs] bf16 (even j in partitions
  0:64, odd in 64:128), chunked 4096 pairs.  Both layers work in 1024-col
  PSUM half-groups (2 banks, double-buffered pools: 2+2 tiles = 8 banks),
  so the in-order engine queues pipeline with two steps of slack.  Layer-2
  work of chunk c-1 interleaves with layer-1 of chunk c.

The f2*x products and the whole j-reduction (7-level f32 add tree +
even/odd combine + corr) run on the otherwise-idle GPSIMD (Pool) engine,
keeping the DVE queue free for the softplus-finish ops that gate PSUM
reuse.  Output stays [d, i] on device; the host transposes back.
"""

import numpy as np
import ml_dtypes

import concourse.bass as bass
import concourse.tile as tile
from concourse import bacc, mybir
from concourse.bass_utils import run_bass_kernel_spmd

LOG2 = float(np.log(2.0))

B, N, D, RBF = 8, 256, 128, 64
PAIRS = N * N // 2            # 32768 row-pairs per batch
CHUNK_PAIRS = 4096            # pairs per DMA-transpose chunk (1 MiB)
GROUP_PAIRS = 1024            # pairs per half-group (8 query nodes i)
SUB = 512                     # cols per matmul (one PSUM bank)
HG = 1024                     # cols per PSUM half-group tile
I_PER_GROUP = GROUP_PAIRS // (N // 2)   # 8
H = CHUNK_PAIRS // GROUP_PAIRS          # groups per chunk tile (4)
N_CORES = 8

BF16 = mybir.dt.bfloat16
F32 = mybir.dt.float32

# Entropy-term fit constants: S(p) ~= d + a*w + c*w*z^2, w = p(1-p).
# Minimax-fitted over the empirical preactivation ranges (layer1 z ~ +-7,
# layer2 z ~ +-4.5); the d's are folded host-side (see module docstring).
D1_C, A1_C, C1_C = -0.00505643, 2.80592749, 0.17756259
D2_C, A2_C, C2_C = -0.01382355, 2.83344796, 0.19336128

_SOFTPLUS_OP = None


def _register_softplus_op():
    """Register the fused softplus-finish DVE op (idempotent).

    out = in0*in1 + w*(s0 + imm2*in0^2),  w = in1*(1-in1)
    """
    global _SOFTPLUS_OP
    if _SOFTPLUS_OP is not None:
        return _SOFTPLUS_OP
    import concourse.dve_ops as dve_ops

    name = "SOFTPLUS_VAR_FIN"
    for op in dve_ops.OPS:
        if op.name == name:
            _SOFTPLUS_OP = op
            return op

    from concourse.dve_ops import DveOp
    from concourse.dve_spec import C0, C2, One, Spec, Src0, Src1, lower, sq
    from concourse.dve_uop import DveOpSpec

    w = Src1 * (One - Src1)
    body = w * (sq(Src0) * C2 + C0) + Src0 * Src1

    def ref(in0, in1, s0, s1, imm2):
        in0 = np.asarray(in0, np.float32)
        in1 = np.asarray(in1, np.float32)
        wv = in1 * (1.0 - in1)
        return wv * (in0 * in0 * imm2 + s0) + in0 * in1

    spec = Spec(body=body, reference=ref)
    row = dve_ops._CUSTOM_DVE_ROW_BASE + len(dve_ops.OPS)
    shas = {}
    for ver in ("v3", "v4"):
        uops = lower(spec, ver=ver)
        shas[ver] = DveOpSpec(
            name=name, opcode=row, uops=uops, rd1_en=True
        ).sha(ver)
    op = DveOp(name, spec, subdim=False, uops_sha=shas)
    dve_ops.OPS.append(op)
    dve_ops._SUB_OPCODE_FOR_NAME[name] = row
    dve_ops.CUSTOM_DVE_SPECS[name] = spec
    _SOFTPLUS_OP = op
    return op


def _build_program(reps: int = 1, unroll: int = 1):
    sp_op = _register_softplus_op()

    nc = bacc.Bacc("TRN2", target_bir_lowering=False, debug=False,
                   num_devices=N_CORES)

    rp = nc.dram_tensor("rp", [PAIRS, 2 * RBF], BF16, kind="ExternalInput").ap()
    xte = nc.dram_tensor("xte", [D, N // 2], BF16, kind="ExternalInput").ap()
    xto = nc.dram_tensor("xto", [D, N // 2], BF16, kind="ExternalInput").ap()
    corr = nc.dram_tensor("corr", [D, 1], F32, kind="ExternalInput").ap()
    w1s = nc.dram_tensor("w1s", [2 * RBF, D], BF16, kind="ExternalInput").ap()
    w2 = nc.dram_tensor("w2", [D, D], BF16, kind="ExternalInput").ap()
    b1c = nc.dram_tensor("b1c", [D, 1], F32, kind="ExternalInput").ap()
    bias2 = nc.dram_tensor("bias2", [2, D], BF16, kind="ExternalInput").ap()
    ones = nc.dram_tensor("ones", [2, SUB], BF16, kind="ExternalInput").ap()
    outT = nc.dram_tensor("outT", [D, N], F32, kind="ExternalOutput").ap()

    f_sig = mybir.ActivationFunctionType.Sigmoid
    mult = mybir.AluOpType.mult

    with tile.TileContext(nc) as tc:
        with (
            tc.tile_pool(name="const", bufs=1) as const,
            tc.tile_pool(name="rt", bufs=3) as rt_pool,
            tc.tile_pool(name="p1", bufs=2) as p1_pool,
            tc.tile_pool(name="p2", bufs=2) as p2_pool,
            tc.tile_pool(name="a1", bufs=2) as a1_pool,
            tc.tile_pool(name="f2", bufs=2) as f2_pool,
            tc.tile_pool(name="prod", bufs=2) as prod_pool,
            tc.tile_pool(name="tree", bufs=1) as tree_pool,
            tc.tile_pool(name="acc", bufs=2) as acc_pool,
            tc.tile_pool(name="osb", bufs=1) as out_pool,
            tc.tile_pool(name="z1", bufs=2, space="PSUM") as z1_pool,
            tc.tile_pool(name="z2", bufs=2, space="PSUM") as z2_pool,
        ):
            w1s_t = const.tile([2 * RBF, D], BF16, tag="w1s")
            w2_t = const.tile([D, D], BF16, tag="w2")
            xte_t = const.tile([D, N // 2], BF16, tag="xte")
            xto_t = const.tile([D, N // 2], BF16, tag="xto")
            b1_t = const.tile([D, 1], F32, tag="b1")
            bias2_t = const.tile([2, D], BF16, tag="bias2")
            ones_t = const.tile([2, SUB], BF16, tag="ones")
            corr_t = const.tile([D, 1], F32, tag="corr")
            nc.sync.dma_start(w1s_t[:], w1s[:])
            nc.sync.dma_start(b1_t[:], b1c[:])
            nc.sync.dma_start(w2_t[:], w2[:])
            nc.sync.dma_start(xte_t[:], xte[:])
            nc.sync.dma_start(xto_t[:], xto[:])
            nc.sync.dma_start(bias2_t[:], bias2[:])
            nc.sync.dma_start(ones_t[:], ones[:])
            nc.sync.dma_start(corr_t[:], corr[:])

            out_sb = out_pool.tile([D, N], F32, tag="osb")

            # Tiny warmup activation right after the const loads: hoists the
            # ~2.7us ACT table load to t~0 where it overlaps the first DMA.
            warm = acc_pool.tile([D, 1], F32, tag="warm")
            nc.scalar.activation(warm[:], b1_t[:], f_sig, bias=0.0)

            jw = N // 2
            PW = H * 2 * HG               # z-cols per chunk tile (8192)
            I_PAIR = H * I_PER_GROUP      # 32 query nodes per chunk tile
            M = PW // jw                  # reduce segments per chunk (64)

            def stage1_half(rt, a1w, hh):
                """mm1 -> sigmoid(+b1) -> softplus-finish for one 1024-col
                half-group (one j-parity of one group) of the current chunk.

                Half-groups + double-buffered 2-bank PSUM pools give the
                mm -> sigma -> finish chains two steps of slack, so the
                in-order engine queues pipeline instead of serializing."""
                h, par = hh // 2, hh % 2
                g0 = h * GROUP_PAIRS
                r0, r1 = par * RBF, (par + 1) * RBF
                z1 = z1_pool.tile([D, HG], F32, tag="z1")
                for s in range(HG // SUB):
                    cs = g0 + s * SUB
                    nc.tensor.matmul(
                        z1[:, s * SUB:(s + 1) * SUB],
                        w1s_t[r0:r1, :],
                        rt[r0:r1, cs:cs + SUB],
                    )
                p1 = p1_pool.tile([D, HG], BF16, tag="p1")
                nc.scalar.activation(p1[:], z1[:], f_sig, bias=b1_t[:])
                nc.vector._custom_dve(
                    sp_op, out=a1w[:, hh * HG:(hh + 1) * HG], in0=z1[:],
                    in1=p1[:], s0=A1_C, imm2=C1_C,
                )

            def stage2_half(a1w, f2w, prod, hh):
                """bias+mm2 -> sigmoid -> softplus-finish -> *x for one
                1024-col half-group of the previous chunk.  The f2*x product
                runs on the Pool engine."""
                c0 = hh * HG
                z2 = z2_pool.tile([D, HG], F32, tag="z2")
                for s in range(HG // SUB):
                    nc.tensor.matmul(
                        z2[:, s * SUB:(s + 1) * SUB],
                        bias2_t[:], ones_t[:],
                        start=True, stop=False, skip_group_check=True,
                    )
                for s in range(HG // SUB):
                    nc.tensor.matmul(
                        z2[:, s * SUB:(s + 1) * SUB],
                        w2_t[:],
                        a1w[:, c0 + s * SUB:c0 + (s + 1) * SUB],
                        start=False, stop=True, skip_group_check=True,
                    )
                p2 = p2_pool.tile([D, HG], BF16, tag="p2")
                nc.scalar.activation(p2[:], z2[:], f_sig, bias=0.0)
                nc.vector._custom_dve(
                    sp_op, out=f2w[:, c0:c0 + HG], in0=z2[:],
                    in1=p2[:], s0=A2_C, imm2=C2_C,
                )
                xb = (xte_t if hh % 2 == 0 else xto_t)[:, None, :]
                nc.vector.tensor_tensor(
                    prod[:, c0:c0 + HG].rearrange("p (k j) -> p k j", j=jw),
                    f2w[:, c0:c0 + HG].rearrange("p (k j) -> p k j", j=jw),
                    xb.broadcast_to([D, I_PER_GROUP, jw]),
                    mult,
                )

            def chunk_tail(prod, i0):
                """j-reduction of one chunk's f2*x products: a 7-level f32
                tensor_add tree + even/odd combine + corr add, all on the
                Pool engine."""
                m3 = prod[:].rearrange("p (m j) -> p m j", j=jw)
                t = m3
                for lvl in range(7):
                    half = jw >> (lvl + 1)
                    nxt = tree_pool.tile([D, M, half], F32, tag=f"t{lvl}")
                    nc.gpsimd.tensor_add(
                        nxt[:], t[:, :, 0:half], t[:, :, half:2 * half])
                    t = nxt
                # t is [D, M, 1]; segments m = h*16 + par*8 + k
                s4 = t[:].rearrange(
                    "p (h par k) o -> p h par (k o)", h=H, par=2)
                tmp = acc_pool.tile([D, I_PAIR], F32, tag="tmp")
                nc.gpsimd.tensor_add(
                    tmp[:].rearrange("p (h k) -> p h k", h=H),
                    s4[:, :, 0, :], s4[:, :, 1, :])
                nc.gpsimd.tensor_scalar_add(
                    out_sb[:, i0:i0 + I_PAIR], tmp[:], corr_t[:])

            def group_tail(prod, g, i0):
                """per-group variant of chunk_tail used in the final flush so
                the tree overlaps the remaining layer-2 work."""
                m3 = prod[:].rearrange("p (m j) -> p m j", j=jw)
                t = m3[:, 16 * g:16 * (g + 1), :]
                for lvl in range(7):
                    half = jw >> (lvl + 1)
                    nxt = tree_pool.tile([D, 16, half], F32, tag=f"g{lvl}")
                    nc.gpsimd.tensor_add(
                        nxt[:], t[:, :, 0:half], t[:, :, half:2 * half])
                    t = nxt
                s4 = t[:].rearrange("p (par k) o -> p par (k o)", par=2)
                tmp = acc_pool.tile([D, I_PER_GROUP], F32, tag="tmpg")
                nc.gpsimd.tensor_add(tmp[:], s4[:, 0, :], s4[:, 1, :])
                nc.gpsimd.tensor_scalar_add(
                    out_sb[:, i0 + 8 * g:i0 + 8 * (g + 1)], tmp[:], corr_t[:])

            # Software-pipelined emission interleaving half-groups of chunk
            # c's layer 1 with half-groups of chunk c-1's layer 2.
            def body():
                pending = None  # (a1w, i0) of the previous chunk
                for c in range(PAIRS // CHUNK_PAIRS):
                    rt = rt_pool.tile([2 * RBF, CHUNK_PAIRS], BF16, tag="rt")
                    if c == 0:
                        # Slice the first transpose 8 ways so mm1 of the
                        # first half-group starts as soon as 128 KiB lands.
                        qq = CHUNK_PAIRS // 8
                        for k in range(8):
                            nc.sync.dma_start_transpose(
                                out=rt[:, k * qq:(k + 1) * qq],
                                in_=rp[k * qq:(k + 1) * qq, :],
                            )
                    else:
                        nc.sync.dma_start_transpose(
                            out=rt[:],
                            in_=rp[c * CHUNK_PAIRS:(c + 1) * CHUNK_PAIRS, :],
                        )
                    a1w = a1_pool.tile([D, PW], BF16, tag="a1")
                    if pending is not None:
                        f2w = f2_pool.tile([D, PW], BF16, tag="f2")
                        prod = prod_pool.tile([D, PW], BF16, tag="prod")
                    for h in range(H):
                        stage1_half(rt, a1w, 2 * h)
                        if pending is not None:
                            stage2_half(pending[0], f2w, prod, 2 * h)
                        stage1_half(rt, a1w, 2 * h + 1)
                        if pending is not None:
                            stage2_half(pending[0], f2w, prod, 2 * h + 1)
                    if pending is not None:
                        chunk_tail(prod, pending[1])
                    pending = (a1w, c * I_PAIR)
                # flush the last chunk's layer 2 (z2 pool double-buffers);
                # per-group tails so the Pool tree overlaps the layer-2 work.
                f2w = f2_pool.tile([D, PW], BF16, tag="f2")
                prod = prod_pool.tile([D, PW], BF16, tag="prod")
                for hh in range(2 * H):
                    stage2_half(pending[0], f2w, prod, hh)
                    if hh % 2 == 1:
                        group_tail(prod, hh // 2, pending[1])

            if unroll > 1:
                for _ in range(unroll):
                    body()
            elif reps == 1:
                body()
            else:
                with tc.For_i(0, reps, 1):
                    body()

            nc.sync.dma_start(outT[:], out_sb[:])

    nc.compile()
    return nc


def _prepare_inputs(x, r, W1, b1, W2, b2):
    bf16 = ml_dtypes.bfloat16
    W1 = np.asarray(W1, np.float32)
    W2 = np.asarray(W2, np.float32)
    W2d = W2.astype(np.float64)
    b1d = np.asarray(b1, np.float64)
    w1s = np.concatenate([W1, W1], axis=0).astype(bf16)          # [128, 128]
    w2b = W2.astype(bf16)                                        # [128, 128]
    # layer-2 bias with: the -log2 shift of layer 1's ssp, layer-1's fit
    # constant d1 (both through W2's column sums), and the mean effect of
    # b1's silu term (0.5 * b1^T W2; bias1 has no PSUM matmul of its own).
    b2n = (np.asarray(b2, np.float64)
           + (D1_C - LOG2) * W2d.sum(axis=0)
           + 0.5 * (b1d @ W2d))

    hi = b2n.astype(np.float32).astype(bf16)
    lo = (b2n - hi.astype(np.float64)).astype(np.float32).astype(bf16)
    bias2 = np.stack([hi, lo])                                   # [2, D]
    ones = np.ones((2, SUB), dtype=bf16)
    b1c = b1d.astype(np.float32).reshape(D, 1)

    in_maps = []
    for b in range(B):
        xbT = np.asarray(x[b], np.float32).T                     # [128 d, 256 j]
        in_maps.append({
            "rp": np.ascontiguousarray(
                np.asarray(r[b], np.float32).reshape(PAIRS, 2 * RBF)
            ).astype(bf16),
            "xte": np.ascontiguousarray(xbT[:, 0::2]).astype(bf16),
            "xto": np.ascontiguousarray(xbT[:, 1::2]).astype(bf16),
            # layer-2's -log2 shift and fit constant d2, scaled by sum_j x.
            "corr": ((D2_C - LOG2) * xbT.sum(axis=1, dtype=np.float64)
                     ).astype(np.float32).reshape(D, 1),
            "w1s": w1s,
            "w2": w2b,
            "b1c": b1c,
            "bias2": bias2,
            "ones": ones,
        })
    return in_maps


_NC_CACHE = None


def _get_nc():
    global _NC_CACHE
    if _NC_CACHE is None:
        _NC_CACHE = _build_program()
    return _NC_CACHE


def hw_time_ns(inputs, reps=2049, n_meas=4):
    """Measure on-device per-iteration time by comparing wall time of a
    reps-times device loop against a single-iteration run."""
    import time as _time
    in_maps = _prepare_inputs(**inputs)

    def run_with(nc_prog):
        ts = []
        for _ in range(n_meas):
            t0 = _time.time()
            run_bass_kernel_spmd(nc_prog, in_maps, list(range(N_CORES)))
            ts.append(_time.time() - t0)
        return min(ts)

    nc1 = _build_program(reps=1)
    ncr = _build_program(reps=reps)
    w1 = run_with(nc1)
    wr = run_with(ncr)
    return (wr - w1) / (reps - 1) * 1e9


def kernel(x, r, W1, b1, W2, b2, _trace=False, _trace_kwargs=None):
    nc = _get_nc()
    in_maps = _prepare_inputs(x, r, W1, b1, W2, b2)
    res = run_bass_kernel_spmd(
        nc, in_maps, list(range(N_CORES)),
        trace=_trace, **(_trace_kwargs or {}),
    )
    out = np.stack([
        np.asarray(res.results[b]["outT"], np.float32).T for b in range(B)
    ])
    if _trace:
        return out, res
    return out


# revision 31
# speedup vs baseline: 1.2119x; 1.0502x over previous
"""CFConv (gnn message passing) Trainium2 kernel.

Math (per batch b):
    f1 = ssp(r @ W1 + b1)            ssp(x) = softplus(x) - log2
    f2 = ssp(f1 @ W2 + b2)
    out[i, d] = sum_j x[j, d] * f2[i, j, d]

Sharding: data-parallel over batch B=8 across the 8 cores (one batch each).

softplus is evaluated in ONE ACT pass + one fused DVE op per layer via the
variational (Legendre) form
    softplus(z) = z*sigma(z) + S(sigma(z)),
    S(p) = -p ln p - (1-p) ln(1-p)   (binary entropy),
which is first-order INSENSITIVE to errors in p = sigma(z) (d/dp vanishes
at p = sigma(z)), so a bf16 p and an approximate S are both safe.  S is
fitted as
    S(p) ~= d + w*(a + c*z^2),  w = p*(1-p)
(the w*z^2 term captures the -w*ln w tails since z ~= -+ln w there).  The
constant d costs nothing on device: layer-1's d folds into layer-2's bias
row and layer-2's d folds into the output correction vector.

Why this beats the classic Exp->Ln two-ACT-pass softplus here: measured on
hardware, an ACT op costs ~1.15us/1024 cols and the fused DVE op ~1.47us,
so sigma+finish = 1.15 ACT + 1.47 DVE per 1024 cols per layer, splitting
the work across two engines (ACT ~147us, DVE ~188us per core) instead of
piling 4 passes on ACT (~260us).  Mixing the two recipes is impossible:
Sigmoid shares no ACT table set with Exp/Ln, and a table switch costs
~2.7us.

The fused DVE op (8 ALU stages, the hardware max):
    out = z*p + w*(C0 + C2*z^2)      in0=z (f32, PSUM), in1=p (bf16, SBUF)
registered at import into dve_ops.OPS; the per-NEFF DVE table mechanism
ships it to the device (no firmware change).

Bias handling (keeps all three custom-op scalar slots free for fit consts):
 - layer 1: b1 (~0.03) enters sigma via the ACT affine; its silu-term
   effect b1*p is folded host-side as 0.5*b1^T W2 into the layer-2 bias
   (residual ~(p-0.5)*b1 is negligible).  mm1 output stays raw.
 - layer 2: bias reaches PSUM exactly via a K=2 ones matmul whose
   stationary holds the bias split hi/lo across two bf16 rows (~4e-6).
End-to-end sim error of the full bf16 pipeline vs the f64 reference:
~1.2e-2 (gate 2e-2).

Per-core pipeline (features on partitions, (i,j)-pairs on the free dim):
  r pairs DMA-transposed to SBUF [128, pairs] bf16 (even j in partitions
  0:64, odd in 64:128), chunked 4096 pairs.  Both layers work in 1024-col
  PSUM half-groups (2 banks, double-buffered pools: 2+2 tiles = 8 banks),
  so the in-order engine queues pipeline with two steps of slack.  Layer-2
  work of chunk c-1 interleaves with layer-1 of chunk c.

The f2*x products and the whole j-reduction (7-level f32 add tree +
even/odd combine + corr) run on the otherwise-idle GPSIMD (Pool) engine,
keeping the DVE queue free for the softplus-finish ops that gate PSUM
reuse.  Output stays [d, i] on device; the host transposes back.
"""

import numpy as np
import ml_dtypes

import concourse.bass as bass
import concourse.tile as tile
from concourse import bacc, mybir
from concourse.bass_utils import run_bass_kernel_spmd

LOG2 = float(np.log(2.0))

B, N, D, RBF = 8, 256, 128, 64
PAIRS = N * N // 2            # 32768 row-pairs per batch
CHUNK_PAIRS = 4096            # pairs per DMA-transpose chunk (1 MiB)
GROUP_PAIRS = 1024            # pairs per half-group (8 query nodes i)
SUB = 512                     # cols per matmul (one PSUM bank)
HG = 1024                     # cols per PSUM half-group tile
I_PER_GROUP = GROUP_PAIRS // (N // 2)   # 8
H = CHUNK_PAIRS // GROUP_PAIRS          # groups per chunk tile (4)
N_CORES = 8

BF16 = mybir.dt.bfloat16
F32 = mybir.dt.float32

# Entropy-term fit constants: S(p) ~= d + a*w + c*w*z^2, w = p(1-p).
# Minimax-fitted over the empirical preactivation ranges (layer1 z ~ +-7,
# layer2 z ~ +-4.5); the d's are folded host-side (see module docstring).
D1_C, A1_C, C1_C = -0.00505643, 2.80592749, 0.17756259
D2_C, A2_C, C2_C = -0.01382355, 2.83344796, 0.19336128

_SOFTPLUS_OP = None


def _register_softplus_op():
    """Register the fused softplus-finish DVE op (idempotent).

    out = in0*in1 + w*(s0 + imm2*in0^2),  w = in1*(1-in1)
    """
    global _SOFTPLUS_OP
    if _SOFTPLUS_OP is not None:
        return _SOFTPLUS_OP
    import concourse.dve_ops as dve_ops

    name = "SOFTPLUS_VAR_FIN"
    for op in dve_ops.OPS:
        if op.name == name:
            _SOFTPLUS_OP = op
            return op

    from concourse.dve_ops import DveOp
    from concourse.dve_spec import C0, C2, One, Spec, Src0, Src1, lower, sq
    from concourse.dve_uop import DveOpSpec

    w = Src1 * (One - Src1)
    body = w * (sq(Src0) * C2 + C0) + Src0 * Src1

    def ref(in0, in1, s0, s1, imm2):
        in0 = np.asarray(in0, np.float32)
        in1 = np.asarray(in1, np.float32)
        wv = in1 * (1.0 - in1)
        return wv * (in0 * in0 * imm2 + s0) + in0 * in1

    spec = Spec(body=body, reference=ref)
    row = dve_ops._CUSTOM_DVE_ROW_BASE + len(dve_ops.OPS)
    shas = {}
    for ver in ("v3", "v4"):
        uops = lower(spec, ver=ver)
        shas[ver] = DveOpSpec(
            name=name, opcode=row, uops=uops, rd1_en=True
        ).sha(ver)
    op = DveOp(name, spec, subdim=False, uops_sha=shas)
    dve_ops.OPS.append(op)
    dve_ops._SUB_OPCODE_FOR_NAME[name] = row
    dve_ops.CUSTOM_DVE_SPECS[name] = spec
    _SOFTPLUS_OP = op
    return op


def _build_program(reps: int = 1, unroll: int = 1):
    sp_op = _register_softplus_op()

    nc = bacc.Bacc("TRN2", target_bir_lowering=False, debug=False,
                   num_devices=N_CORES)

    rp = nc.dram_tensor("rp", [PAIRS, 2 * RBF], BF16, kind="ExternalInput").ap()
    xte = nc.dram_tensor("xte", [D, N // 2], BF16, kind="ExternalInput").ap()
    xto = nc.dram_tensor("xto", [D, N // 2], BF16, kind="ExternalInput").ap()
    corr = nc.dram_tensor("corr", [D, 1], F32, kind="ExternalInput").ap()
    w1s = nc.dram_tensor("w1s", [2 * RBF, D], BF16, kind="ExternalInput").ap()
    w2 = nc.dram_tensor("w2", [D, D], BF16, kind="ExternalInput").ap()
    b1c = nc.dram_tensor("b1c", [D, 1], F32, kind="ExternalInput").ap()
    bias2 = nc.dram_tensor("bias2", [2, D], BF16, kind="ExternalInput").ap()
    ones = nc.dram_tensor("ones", [2, SUB], BF16, kind="ExternalInput").ap()
    outT = nc.dram_tensor("outT", [D, N], F32, kind="ExternalOutput").ap()

    f_sig = mybir.ActivationFunctionType.Sigmoid
    mult = mybir.AluOpType.mult

    with tile.TileContext(nc) as tc:
        with (
            tc.tile_pool(name="const", bufs=1) as const,
            tc.tile_pool(name="rt", bufs=3) as rt_pool,
            tc.tile_pool(name="p1", bufs=2) as p1_pool,
            tc.tile_pool(name="p2", bufs=2) as p2_pool,
            tc.tile_pool(name="a1", bufs=2) as a1_pool,
            tc.tile_pool(name="f2", bufs=2) as f2_pool,
            tc.tile_pool(name="prod", bufs=2) as prod_pool,
            tc.tile_pool(name="tree", bufs=1) as tree_pool,
            tc.tile_pool(name="acc", bufs=2) as acc_pool,
            tc.tile_pool(name="osb", bufs=1) as out_pool,
            tc.tile_pool(name="z1", bufs=2, space="PSUM") as z1_pool,
            tc.tile_pool(name="z2", bufs=2, space="PSUM") as z2_pool,
        ):
            w1s_t = const.tile([2 * RBF, D], BF16, tag="w1s")
            w2_t = const.tile([D, D], BF16, tag="w2")
            xte_t = const.tile([D, N // 2], BF16, tag="xte")
            xto_t = const.tile([D, N // 2], BF16, tag="xto")
            b1_t = const.tile([D, 1], F32, tag="b1")
            bias2_t = const.tile([2, D], BF16, tag="bias2")
            ones_t = const.tile([2, SUB], BF16, tag="ones")
            corr_t = const.tile([D, 1], F32, tag="corr")
            # Critical-path consts first: mm1 of the first half-group needs
            # only w1s (+ the first r piece, issued at the top of body()).
            nc.sync.dma_start(w1s_t[:], w1s[:])
            nc.sync.dma_start(b1_t[:], b1c[:])

            out_sb = out_pool.tile([D, N], F32, tag="osb")

            # Tiny warmup activation right after the const loads: hoists the
            # ~2.7us ACT table load to t~0 where it overlaps the first DMA.
            warm = acc_pool.tile([D, 1], F32, tag="warm")
            nc.scalar.activation(warm[:], b1_t[:], f_sig, bias=0.0)

            jw = N // 2
            PW = H * 2 * HG               # z-cols per chunk tile (8192)
            I_PAIR = H * I_PER_GROUP      # 32 query nodes per chunk tile
            M = PW // jw                  # reduce segments per chunk (64)

            def stage1_half(rt, a1w, hh):
                """mm1 -> sigmoid(+b1) -> softplus-finish for one 1024-col
                half-group (one j-parity of one group) of the current chunk.

                Half-groups + double-buffered 2-bank PSUM pools give the
                mm -> sigma -> finish chains two steps of slack, so the
                in-order engine queues pipeline instead of serializing."""
                h, par = hh // 2, hh % 2
                g0 = h * GROUP_PAIRS
                r0, r1 = par * RBF, (par + 1) * RBF
                z1 = z1_pool.tile([D, HG], F32, tag="z1")
                for s in range(HG // SUB):
                    cs = g0 + s * SUB
                    nc.tensor.matmul(
                        z1[:, s * SUB:(s + 1) * SUB],
                        w1s_t[r0:r1, :],
                        rt[r0:r1, cs:cs + SUB],
                    )
                p1 = p1_pool.tile([D, HG], BF16, tag="p1")
                nc.scalar.activation(p1[:], z1[:], f_sig, bias=b1_t[:])
                nc.vector._custom_dve(
                    sp_op, out=a1w[:, hh * HG:(hh + 1) * HG], in0=z1[:],
                    in1=p1[:], s0=A1_C, imm2=C1_C,
                )

            def stage2_half(a1w, f2w, prod, hh):
                """bias+mm2 -> sigmoid -> softplus-finish -> *x for one
                1024-col half-group of the previous chunk.  The f2*x product
                runs on the Pool engine."""
                c0 = hh * HG
                z2 = z2_pool.tile([D, HG], F32, tag="z2")
                for s in range(HG // SUB):
                    nc.tensor.matmul(
                        z2[:, s * SUB:(s + 1) * SUB],
                        bias2_t[:], ones_t[:],
                        start=True, stop=False, skip_group_check=True,
                    )
                for s in range(HG // SUB):
                    nc.tensor.matmul(
                        z2[:, s * SUB:(s + 1) * SUB],
                        w2_t[:],
                        a1w[:, c0 + s * SUB:c0 + (s + 1) * SUB],
                        start=False, stop=True, skip_group_check=True,
                    )
                p2 = p2_pool.tile([D, HG], BF16, tag="p2")
                nc.scalar.activation(p2[:], z2[:], f_sig, bias=0.0)
                nc.vector._custom_dve(
                    sp_op, out=f2w[:, c0:c0 + HG], in0=z2[:],
                    in1=p2[:], s0=A2_C, imm2=C2_C,
                )
                xb = (xte_t if hh % 2 == 0 else xto_t)[:, None, :]
                nc.vector.tensor_tensor(
                    prod[:, c0:c0 + HG].rearrange("p (k j) -> p k j", j=jw),
                    f2w[:, c0:c0 + HG].rearrange("p (k j) -> p k j", j=jw),
                    xb.broadcast_to([D, I_PER_GROUP, jw]),
                    mult,
                )

            def chunk_tail(prod, i0):
                """j-reduction of one chunk's f2*x products: a 7-level f32
                tensor_add tree + even/odd combine + corr add, all on the
                Pool engine."""
                m3 = prod[:].rearrange("p (m j) -> p m j", j=jw)
                t = m3
                for lvl in range(7):
                    half = jw >> (lvl + 1)
                    nxt = tree_pool.tile([D, M, half], F32, tag=f"t{lvl}")
                    nc.gpsimd.tensor_add(
                        nxt[:], t[:, :, 0:half], t[:, :, half:2 * half])
                    t = nxt
                # t is [D, M, 1]; segments m = h*16 + par*8 + k
                s4 = t[:].rearrange(
                    "p (h par k) o -> p h par (k o)", h=H, par=2)
                tmp = acc_pool.tile([D, I_PAIR], F32, tag="tmp")
                nc.gpsimd.tensor_add(
                    tmp[:].rearrange("p (h k) -> p h k", h=H),
                    s4[:, :, 0, :], s4[:, :, 1, :])
                nc.gpsimd.tensor_scalar_add(
                    out_sb[:, i0:i0 + I_PAIR], tmp[:], corr_t[:])
                nc.sync.dma_start(
                    outT[:, i0:i0 + I_PAIR], out_sb[:, i0:i0 + I_PAIR])

            def group_tail(prod, g, i0):
                """per-group variant of chunk_tail used in the final flush so
                the tree overlaps the remaining layer-2 work."""
                m3 = prod[:].rearrange("p (m j) -> p m j", j=jw)
                t = m3[:, 16 * g:16 * (g + 1), :]
                for lvl in range(7):
                    half = jw >> (lvl + 1)
                    nxt = tree_pool.tile([D, 16, half], F32, tag=f"g{lvl}")
                    nc.gpsimd.tensor_add(
                        nxt[:], t[:, :, 0:half], t[:, :, half:2 * half])
                    t = nxt
                s4 = t[:].rearrange("p (par k) o -> p par (k o)", par=2)
                tmp = acc_pool.tile([D, I_PER_GROUP], F32, tag="tmpg")
                nc.gpsimd.tensor_add(tmp[:], s4[:, 0, :], s4[:, 1, :])
                nc.gpsimd.tensor_scalar_add(
                    out_sb[:, i0 + 8 * g:i0 + 8 * (g + 1)], tmp[:], corr_t[:])
                nc.sync.dma_start(
                    outT[:, i0 + 8 * g:i0 + 8 * (g + 1)],
                    out_sb[:, i0 + 8 * g:i0 + 8 * (g + 1)])

            # Software-pipelined emission interleaving half-groups of chunk
            # c's layer 1 with half-groups of chunk c-1's layer 2.
            def body():
                pending = None  # (a1w, i0) of the previous chunk
                for c in range(PAIRS // CHUNK_PAIRS):
                    rt = rt_pool.tile([2 * RBF, CHUNK_PAIRS], BF16, tag="rt")
                    if c == 0:
                        # Slice the first transpose 8 ways so mm1 of the
                        # first half-group starts as soon as 128 KiB lands;
                        # the remaining (non-critical) consts load after the
                        # first two slices.
                        qq = CHUNK_PAIRS // 8
                        for k in range(8):
                            nc.sync.dma_start_transpose(
                                out=rt[:, k * qq:(k + 1) * qq],
                                in_=rp[k * qq:(k + 1) * qq, :],
                            )
                            if k == 1:
                                nc.sync.dma_start(w2_t[:], w2[:])
                                nc.sync.dma_start(bias2_t[:], bias2[:])
                                nc.sync.dma_start(ones_t[:], ones[:])
                                nc.sync.dma_start(xte_t[:], xte[:])
                                nc.sync.dma_start(xto_t[:], xto[:])
                                nc.sync.dma_start(corr_t[:], corr[:])
                    else:
                        nc.sync.dma_start_transpose(
                            out=rt[:],
                            in_=rp[c * CHUNK_PAIRS:(c + 1) * CHUNK_PAIRS, :],
                        )
                    a1w = a1_pool.tile([D, PW], BF16, tag="a1")
                    if pending is not None:
                        f2w = f2_pool.tile([D, PW], BF16, tag="f2")
                        prod = prod_pool.tile([D, PW], BF16, tag="prod")
                    for h in range(H):
                        stage1_half(rt, a1w, 2 * h)
                        if pending is not None:
                            stage2_half(pending[0], f2w, prod, 2 * h)
                        stage1_half(rt, a1w, 2 * h + 1)
                        if pending is not None:
                            stage2_half(pending[0], f2w, prod, 2 * h + 1)
                    if pending is not None:
                        chunk_tail(prod, pending[1])
                    pending = (a1w, c * I_PAIR)
                # flush the last chunk's layer 2 (z2 pool double-buffers);
                # per-group tails so the Pool tree overlaps the layer-2 work.
                f2w = f2_pool.tile([D, PW], BF16, tag="f2")
                prod = prod_pool.tile([D, PW], BF16, tag="prod")
                for hh in range(2 * H):
                    stage2_half(pending[0], f2w, prod, hh)
                    if hh % 2 == 1:
                        group_tail(prod, hh // 2, pending[1])

            if unroll > 1:
                for _ in range(unroll):
                    body()
            elif reps == 1:
                body()
            else:
                with tc.For_i(0, reps, 1):
                    body()

    nc.compile()
    return nc


def _prepare_inputs(x, r, W1, b1, W2, b2):
    bf16 = ml_dtypes.bfloat16
    W1 = np.asarray(W1, np.float32)
    W2 = np.asarray(W2, np.float32)
    W2d = W2.astype(np.float64)
    b1d = np.asarray(b1, np.float64)
    w1s = np.concatenate([W1, W1], axis=0).astype(bf16)          # [128, 128]
    w2b = W2.astype(bf16)                                        # [128, 128]
    # layer-2 bias with: the -log2 shift of layer 1's ssp, layer-1's fit
    # constant d1 (both through W2's column sums), and the mean effect of
    # b1's silu term (0.5 * b1^T W2; bias1 has no PSUM matmul of its own).
    b2n = (np.asarray(b2, np.float64)
           + (D1_C - LOG2) * W2d.sum(axis=0)
           + 0.5 * (b1d @ W2d))

    hi = b2n.astype(np.float32).astype(bf16)
    lo = (b2n - hi.astype(np.float64)).astype(np.float32).astype(bf16)
    bias2 = np.stack([hi, lo])                                   # [2, D]
    ones = np.ones((2, SUB), dtype=bf16)
    b1c = b1d.astype(np.float32).reshape(D, 1)

    in_maps = []
    for b in range(B):
        xbT = np.asarray(x[b], np.float32).T                     # [128 d, 256 j]
        in_maps.append({
            "rp": np.ascontiguousarray(
                np.asarray(r[b], np.float32).reshape(PAIRS, 2 * RBF)
            ).astype(bf16),
            "xte": np.ascontiguousarray(xbT[:, 0::2]).astype(bf16),
            "xto": np.ascontiguousarray(xbT[:, 1::2]).astype(bf16),
            # layer-2's -log2 shift and fit constant d2, scaled by sum_j x.
            "corr": ((D2_C - LOG2) * xbT.sum(axis=1, dtype=np.float64)
                     ).astype(np.float32).reshape(D, 1),
            "w1s": w1s,
            "w2": w2b,
            "b1c": b1c,
            "bias2": bias2,
            "ones": ones,
        })
    return in_maps


_NC_CACHE = None


def _get_nc():
    global _NC_CACHE
    if _NC_CACHE is None:
        _NC_CACHE = _build_program()
    return _NC_CACHE


def hw_time_ns(inputs, reps=2049, n_meas=4):
    """Measure on-device per-iteration time by comparing wall time of a
    reps-times device loop against a single-iteration run."""
    import time as _time
    in_maps = _prepare_inputs(**inputs)

    def run_with(nc_prog):
        ts = []
        for _ in range(n_meas):
            t0 = _time.time()
            run_bass_kernel_spmd(nc_prog, in_maps, list(range(N_CORES)))
            ts.append(_time.time() - t0)
        return min(ts)

    nc1 = _build_program(reps=1)
    ncr = _build_program(reps=reps)
    w1 = run_with(nc1)
    wr = run_with(ncr)
    return (wr - w1) / (reps - 1) * 1e9


def kernel(x, r, W1, b1, W2, b2, _trace=False, _trace_kwargs=None):
    nc = _get_nc()
    in_maps = _prepare_inputs(x, r, W1, b1, W2, b2)
    res = run_bass_kernel_spmd(
        nc, in_maps, list(range(N_CORES)),
        trace=_trace, **(_trace_kwargs or {}),
    )
    out = np.stack([
        np.asarray(res.results[b]["outT"], np.float32).T for b in range(B)
    ])
    if _trace:
        return out, res
    return out
